# revision 14
# baseline (speedup 1.0000x reference)
"""Multi-head attention (B=2, S=4096, D=768, H=12) on 8 Trainium2 cores.

Sharding: batch x heads. Core c = (b, r) with b = c // 4, r = c % 4 handles
batch b and heads {3r, 3r+1, 3r+2}.

v3 design (vs v2): the softmax exp stream is split across TWO engines --
ACT does LUT exp on the first 512 score columns of each [128,1024] PSUM
tile, DVE does a Schraudolph bit-trick exp on the other 512 (one fused
tensor_scalar mult-add writing int16 "bf16 bits", bitcast to bf16; rel
err ~1.5% rms, washes out in the softmax average). The normalization
chain drops the PE broadcast matmul and two DVE copies: reciprocal runs
directly on the PSUM sum row (partition 64, lane-aligned), GPSIMD
broadcasts it across partitions, one DVE multiply normalizes. The output
bias bo is folded into the out-projection as a 65th contraction row on
head j2 (ones row in oT2, bo/4 row in Wo), so the phase-3 PSUM
evacuation is a plain ACT copy. Phase-1 Q/K bias-add evacuations moved
to the otherwise-idle ACT engine (Identity activation with per-partition
bias); the exp/identity/copy funcs share one ACT table (no reloads).

Per core:
  phase 1: load x (bf16), project Q^T/K^T per head into [128, S] tiles
           (head data duplicated across partition halves via sbuf DMA),
           V' = [V+bv | 1] in [128, 32, 195] chunk layout.
  phase 2: per (head, 512-query supertile): 16 iterations, each: two
           128-key score matmuls (row-tiled halves) into one [128,1024]
           PSUM tile, exp split ACT/DVE -> P^T bf16, two attnV matmuls
           accumulating [65, 512] out'^T + sum row. Normalize via
           reciprocal + GPSIMD partition broadcast + DVE multiply.
  phase 3: out-projection per head (K=64; K=65 with ones/bias row on
           j2), ACT copy evacuation, staged ReduceScatter(add) over the
           4 cores of the batch.
Host: slices weights per core (bf16), reassembles row quarters.
"""
import numpy as np

B, S, D, NH, HD = 2, 4096, 768, 12, 64
P = 128
NCORES = 8
SCALE = 0.125  # 1/sqrt(64)
# Schraudolph exp in bf16-bit domain: bits = trunc(raw_score * A + B).
# A = SCALE * 128 * log2(e); B = 16256 - c with c calibrated offline
# (study_numerics.py: full-kernel rel err 6.1e-3 vs 2e-2 budget).
SCH_A = 23.08312065
SCH_B = 16249.0

_CACHE = {}


def _build(bench_reps=1, loop_reps=None, no_rs=False):
    import concourse.bacc as bacc
    import concourse.mybir as mybir
    import concourse.tile as tile

    F32 = mybir.dt.float32
    BF16 = mybir.dt.bfloat16

    nc = bacc.Bacc("TRN2", target_bir_lowering=False, debug=False,
                   num_devices=NCORES)

    xt_d = nc.dram_tensor("xt", [D, S], BF16, kind="ExternalInput")
    wqk_d = nc.dram_tensor("wqk", [D, 384], BF16, kind="ExternalInput")
    bqk_d = nc.dram_tensor("bqk", [P, 4], F32, kind="ExternalInput")
    wv_d = nc.dram_tensor("wv", [D, 256], BF16, kind="ExternalInput")
    bvpb_d = nc.dram_tensor("bvpb", [P, 256], F32, kind="ExternalInput")
    wo_d = nc.dram_tensor("wo", [195, D], BF16, kind="ExternalInput")
    y_d = nc.dram_tensor("y", [4, 256, D], F32, kind="ExternalOutput")

    with tile.TileContext(nc) as tc:
        cst = tc.alloc_tile_pool(name="cst", bufs=1)
        per = tc.alloc_tile_pool(name="per", bufs=1)
        dram = tc.alloc_tile_pool(name="dram", bufs=1, space="DRAM")

        ones64b = cst.tile([1, 64], BF16)
        nc.vector.memset(ones64b[:], 1.0)

        bqk_s = cst.tile([P, 4], F32)
        nc.sync.dma_start(bqk_s[:], bqk_d[:])
        wqk_s = cst.tile([P, 6, 384], BF16)
        wv_s = cst.tile([P, 6, 256], BF16)
        wo_s = cst.tile([65, 3, D], BF16)
        bvpb_s = cst.tile([P, 256], F32)
        nc.sync.dma_start(wqk_s[:], wqk_d.rearrange("(o p) m -> p o m", p=P))
        nc.sync.dma_start(wv_s[:], wv_d.rearrange("(o p) m -> p o m", p=P))
        nc.sync.dma_start(wo_s[:], wo_d.rearrange("(j p) n -> p j n", p=65))
        nc.sync.dma_start(bvpb_s[:], bvpb_d[:])

        # persistent per-core tensors: per-head Q^T/K^T with the head's data
        # duplicated across both 64-partition halves (for PE row tiling)
        qd = [per.tile([P, S], BF16, name=f"qd{j}") for j in range(3)]
        kd = [per.tile([P, S], BF16, name=f"kd{j}") for j in range(3)]
        vp = per.tile([P, 32, 195], BF16)  # V' chunks: [V_h+bv | 1] at 65*j

        if loop_reps is None:
            for _rep in range(bench_reps):
                _phases(nc, tc, tile, mybir, xt_d, y_d, dram, bqk_s,
                        wqk_s, wv_s, wo_s, bvpb_s, qd, kd, vp,
                        rs_inline=not no_rs, ones64b=ones64b)
        else:
            with tc.For_i(0, loop_reps, 1):
                _phases(nc, tc, tile, mybir, xt_d, y_d, dram, bqk_s,
                        wqk_s, wv_s, wo_s, bvpb_s, qd, kd, vp,
                        rs_inline=False, cc_sink=True, ones64b=ones64b)

        dram.release()
        per.release()
        cst.release()

    nc.compile()
    return nc


def _phases(nc, tc, tile, mybir, xt_d, y_d, dram, bqk_s,
            wqk_s, wv_s, wo_s, bvpb_s, qd, kd, vp,
            rs_inline=True, cc_sink=True, ones64b=None):
    F32 = mybir.dt.float32
    BF16 = mybir.dt.bfloat16
    I16 = mybir.dt.int16
    AF = mybir.ActivationFunctionType
    Alu = mybir.AluOpType
    P = 128
    S, D = 4096, 768
    SCALE = 0.125

    # ---- phase 1: x^T, projections ----
    with (
        tc.tile_pool(name="p1", bufs=2) as p1,
        tc.tile_pool(name="p1ps", bufs=1, space="PSUM") as p1ps,
    ):
        # (group col base, dst tile low half, dst tile high half,
        #  bias col low, bias col high)
        groups = [(0, qd[0], qd[1], 0, 0), (128, kd[0], kd[1], 1, 1),
                  (256, qd[2], kd[2], 2, 2)]
        for sc in range(8):
            cb = slice(512 * sc, 512 * (sc + 1))
            xts = p1.tile([P, 6, 512], BF16, tag="xts")
            nc.sync.dma_start(
                xts[:],
                xt_d[:, 512 * sc:512 * (sc + 1)].rearrange(
                    "(f p) q -> p f q", p=P))
            for gcb, dstL, dstH, bcL, bcH in groups:
                qk_ps = p1ps.tile([P, 512], F32, tag="qkps", bufs=2)
                for f in range(6):
                    nc.tensor.matmul(qk_ps[:], wqk_s[:, f, gcb:gcb + P],
                                     xts[:, f, :],
                                     start=(f == 0), stop=(f == 5))
                # bias-add evacuations on ACT (per-partition scalar bias)
                nc.scalar.activation(dstL[0:64, cb], qk_ps[0:64, :],
                                     AF.Identity,
                                     bias=bqk_s[0:64, bcL:bcL + 1])
                nc.scalar.activation(dstH[64:128, cb], qk_ps[64:128, :],
                                     AF.Identity,
                                     bias=bqk_s[64:128, bcH:bcH + 1])
                # duplicate across partition halves (DMA sbuf->sbuf)
                nc.sync.dma_start(dstL[64:128, cb], dstL[0:64, cb])
                nc.sync.dma_start(dstH[0:64, cb], dstH[64:128, cb])
            for j in range(4):
                v_ps = p1ps.tile([P, 256], F32, tag="vps", bufs=2)
                for f in range(6):
                    nc.tensor.matmul(v_ps[:], xts[:, f, P * j:P * (j + 1)],
                                     wv_s[:, f, :],
                                     start=(f == 0), stop=(f == 5))
                nc.vector.tensor_tensor(vp[:, 4 * sc + j, :],
                                        v_ps[:, 0:195], bvpb_s[:, 0:195],
                                        Alu.add)

    # ---- phase 2+3: attention, out-projection, reduce-scatter ----
    with (
        tc.tile_pool(name="p2", bufs=1) as p2,
        tc.tile_pool(name="p2ps", bufs=1, space="PSUM") as p2ps,
    ):
        pending = []  # deferred out-projection steps (closures)

        def drain_one():
            if pending:
                pending.pop(0)()

        for qg in range(4):
            cc_in = dram.tile([1024, D], F32, tag="ccin", bufs=2)
            for q2 in range(2):
                qs = 2 * qg + q2
                qoff = 512 * qs
                oTs = []
                for j in range(3):
                    o_a = p2ps.tile([65, 512], F32, tag="ops", bufs=2,
                                    name=f"oa{qs}_{j}")
                    # attnV lags the exp by TWO iterations so the PE never
                    # waits on the exp engines (scores_i + attnV_{i-2} give
                    # the split exp of iteration i-1 a full PE-busy window)
                    lagq = []
                    for i in range(16):
                        s_ps = p2ps.tile([P, 1024], F32, tag="sps", bufs=2)
                        # two 128-key score chunks (row-tiled halves)
                        nc.tensor.matmul(
                            s_ps[:, 0:512],
                            kd[j][0:64, 256 * i:256 * i + 128],
                            qd[j][0:64, qoff:qoff + 512],
                            start=True, stop=True)
                        nc.tensor.matmul(
                            s_ps[:, 512:1024],
                            kd[j][64:128, 256 * i + 128:256 * i + 256],
                            qd[j][64:128, qoff:qoff + 512],
                            start=True, stop=True)
                        # exp split: DVE Schraudolph (int16 bf16-bits) on
                        # chunk A -- it only waits on the FIRST score matmul
                        # (subtile dep), so the slower engine starts a slot
                        # early and the sps buffer frees sooner -- ACT LUT
                        # exp on chunk B. Separate tiles so the two writers
                        # never share a tile (the bitcast view defeats
                        # subtile range tracking and would serialize them).
                        ptA = p2.tile([P, 512], BF16, tag="ptA", bufs=4)
                        ptB = p2.tile([P, 512], BF16, tag="ptB", bufs=4)
                        nc.vector.tensor_scalar(
                            ptA.bitcast(I16)[:], s_ps[:, 0:512],
                            SCH_A, SCH_B, Alu.mult, Alu.add)
                        nc.scalar.activation(ptB[:], s_ps[:, 512:1024],
                                             AF.Exp, scale=SCALE)
                        drain_one()
                        lagq.append(((ptA, ptB), i))
                        if len(lagq) > 2:
                            pp, pi = lagq.pop(0)
                            _attnv(nc, vp, pp, o_a, j, pi)
                    for pp, pi in lagq:
                        _attnv(nc, vp, pp, o_a, j, pi)
                    # normalize (v2-style chain, norm_mode="v2"):
                    # evacuate, DMA sum row to partition 0, reciprocal,
                    # PE ones-broadcast, DVE multiply
                    oAll = p2.tile([65, 512], F32, tag="oAll", bufs=2)
                    nc.vector.tensor_copy(oAll[:], o_a[:])
                    srow0 = p2.tile([1, 512], F32, tag="srow0", bufs=2)
                    nc.sync.dma_start(srow0[:], oAll[64:65, :])
                    recip = p2.tile([1, 512], F32, tag="recip", bufs=2)
                    nc.vector.reciprocal_approx_fast(recip[:], srow0[:])
                    recipb = p2.tile([1, 512], BF16, tag="recipb", bufs=2)
                    nc.vector.tensor_copy(recipb[:], recip[:])
                    b_ps = p2ps.tile([65, 512], F32, tag="ops", bufs=2,
                                     name=f"bps{qs}_{j}")
                    nc.tensor.matmul(b_ps[0:64, :], ones64b[:], recipb[:],
                                     start=True, stop=True)
                    if j == 2:
                        oT = p2.tile([65, 512], BF16, tag="oT2", bufs=3)
                        nc.gpsimd.memset(oT[64:65, :], 1.0)
                    else:
                        oT = p2.tile([64, 512], BF16, tag="oT", bufs=6)
                    nc.vector.tensor_tensor(oT[0:64, :], oAll[0:64, :],
                                            b_ps[0:64, :], Alu.mult)
                    oTs.append(oT)

                # deferred out-projection for this qs: one PE matmul per
                # drained step so the exp stream never starves
                def make_outproj(oTs, cc_in, q2):
                    state = {}

                    def mm(t, j, half):
                        def step():
                            if j == 0 and half == 0:
                                state[t] = p2ps.tile([P, D], F32, tag="fps",
                                                     name=f"fps{q2}_{t}")
                            lo, hi = (0, 512) if half == 0 else (512, D)
                            kk = 65 if j == 2 else 64
                            nc.tensor.matmul(
                                state[t][:, lo:hi],
                                oTs[j][0:kk, P * t:P * (t + 1)],
                                wo_s[0:kk, j, lo:hi],
                                start=(j == 0), stop=(j == 2))
                        return step

                    def evac(t):
                        def step():
                            fout = p2.tile([P, D], F32, tag="fout", bufs=3)
                            nc.scalar.copy(fout[:], state[t][:])
                            nc.sync.dma_start(
                                cc_in[512 * q2 + P * t:
                                      512 * q2 + P * (t + 1), :],
                                fout[:])
                        return step

                    steps = []
                    for t in range(4):
                        for j in range(3):
                            for half in range(2):
                                steps.append(mm(t, j, half))
                        steps.append(evac(t))
                    return steps

                pending.extend(make_outproj(oTs, cc_in, q2))

            if rs_inline:
                def make_rs(cc_in, qg):
                    def step():
                        cc_out = dram.tile([256, D], F32, tag="ccout",
                                           bufs=2)
                        nc.gpsimd.collective_compute(
                            "ReduceScatter", mybir.AluOpType.add,
                            replica_groups=[[0, 1, 2, 3], [4, 5, 6, 7]],
                            ins=[cc_in.opt()], outs=[cc_out.opt()])
                        nc.sync.dma_start(y_d[qg], cc_out[:])
                    return step
                pending.append(make_rs(cc_in, qg))
            elif cc_sink:
                def make_sink(cc_in, qg):
                    def step():
                        nc.sync.dma_start(y_d[qg], cc_in[0:256, :])
                    return step
                pending.append(make_sink(cc_in, qg))

        while pending:
            pending.pop(0)()


def _attnv(nc, vp, pts, o_a, j, i):
    # full-K attnV: one matmul per 128-key chunk (chunks 2i and 2i+1)
    for t in range(2):
        kc = 2 * i + t
        nc.tensor.matmul(o_a[:], vp[:, kc, 65 * j:65 * (j + 1)],
                         pts[t][:],
                         start=(kc == 0), stop=(kc == 31))


def _build_loop(loop_reps):
    return _build(loop_reps=loop_reps)


def _get_nc(bench_reps=1):
    key = ("nc", bench_reps)
    if key not in _CACHE:
        if isinstance(bench_reps, tuple) and bench_reps[0] == "loop":
            _CACHE[key] = _build_loop(bench_reps[1])
        else:
            _CACHE[key] = _build(bench_reps)
    return _CACHE[key]


def _make_in_maps(x, Wq, bq, Wk, bk, Wv, bv, Wo, bo):
    import ml_dtypes
    BF = ml_dtypes.bfloat16
    in_maps = []
    for c in range(NCORES):
        b, r = divmod(c, 4)
        hs = [3 * r, 3 * r + 1, 3 * r + 2]
        col = lambda W, h: W[:, HD * h:HD * (h + 1)]
        seg = lambda v, h: v[HD * h:HD * (h + 1)]

        # groups: Q0|Q1 (0:128), K0|K1 (128:256), Q2|K2 (256:384)
        wqk = np.zeros((D, 384), np.float32)
        wqk[:, 0:64] = col(Wq, hs[0]); wqk[:, 64:128] = col(Wq, hs[1])
        wqk[:, 128:192] = col(Wk, hs[0]); wqk[:, 192:256] = col(Wk, hs[1])
        wqk[:, 256:320] = col(Wq, hs[2]); wqk[:, 320:384] = col(Wk, hs[2])

        bqk = np.zeros((P, 4), np.float32)
        bqk[0:64, 0] = seg(bq, hs[0]); bqk[64:128, 0] = seg(bq, hs[1])
        bqk[0:64, 1] = seg(bk, hs[0]); bqk[64:128, 1] = seg(bk, hs[1])
        bqk[0:64, 2] = seg(bq, hs[2]); bqk[64:128, 2] = seg(bk, hs[2])

        wv = np.zeros((D, 256), np.float32)
        bvp = np.zeros((1, 256), np.float32)
        for j in range(3):
            wv[:, 65 * j:65 * j + 64] = col(Wv, hs[j])
            bvp[0, 65 * j:65 * j + 64] = seg(bv, hs[j])
            bvp[0, 65 * j + 64] = 1.0

        # out-projection weights with a 65th bias row on head j2:
        # rows 65j:65j+64 = Wo rows of head j, row 65j+64 = bo/4 (j==2)
        wo = np.zeros((195, D), np.float32)
        for j in range(3):
            wo[65 * j:65 * j + 64, :] = Wo[HD * hs[j]:HD * (hs[j] + 1), :]
        wo[194, :] = bo * 0.25  # summed by 4 cores in the ReduceScatter

        in_maps.append({
            "xt": np.ascontiguousarray(x[b].T).astype(BF),
            "wqk": wqk.astype(BF), "bqk": bqk,
            "wv": wv.astype(BF),
            "bvpb": np.broadcast_to(bvp, (P, 256)).astype(np.float32).copy(),
            "wo": wo.astype(BF),
        })
    return in_maps


def _assemble(results):
    out = np.zeros((B, S, D), np.float32)
    for c in range(NCORES):
        b, r = divmod(c, 4)
        y = results[c]["y"]
        for g in range(4):
            out[b, 1024 * g + 256 * r:1024 * g + 256 * (r + 1), :] = y[g]
    return out


def kernel(x, Wq, bq, Wk, bk, Wv, bv, Wo, bo):
    from concourse.bass_utils import run_bass_kernel_spmd
    args = [np.asarray(a, np.float32) for a in
            (x, Wq, bq, Wk, bk, Wv, bv, Wo, bo)]
    nc = _get_nc()
    in_maps = _make_in_maps(*args)
    res = run_bass_kernel_spmd(nc, in_maps, core_ids=list(range(NCORES)))
    return _assemble(res.results)


# ---------------------------------------------------------------------------
# Timing support (used by test.py, not by the grading path).
def _runner(bench_reps=1):
    import jax
    import numpy as _np
    import concourse.mybir as mybir
    from jax.sharding import Mesh, PartitionSpec
    from jax.experimental.shard_map import shard_map
    from concourse.bass2jax import (_bass_exec_p, install_neuronx_cc_hook,
                                    partition_id_tensor)

    install_neuronx_cc_hook()
    nc = _get_nc(bench_reps)

    partition_name = (nc.partition_id_tensor.name
                      if nc.partition_id_tensor else None)
    in_names, out_names, out_avals = [], [], []
    for alloc in nc.m.functions[0].allocations:
        if not isinstance(alloc, mybir.MemoryLocationSet):
            continue
        name = alloc.memorylocations[0].name
        if alloc.kind == "ExternalInput":
            if name != partition_name:
                in_names.append(name)
        elif alloc.kind == "ExternalOutput":
            out_names.append(name)
            out_avals.append(jax.core.ShapedArray(
                tuple(alloc.tensor_shape), mybir.dt.np(alloc.dtype)))
    n_params = len(in_names)
    all_names = in_names + out_names
    if partition_name is not None:
        all_names.append(partition_name)

    def _body(*args):
        ins = list(args[:n_params])
        outs = list(args[n_params:])
        extra = ([partition_id_tensor()] if partition_name is not None else [])
        outs = list(_bass_exec_p.bind(
            *ins, *outs, *extra,
            out_avals=tuple(out_avals),
            in_names=tuple(all_names),
            out_names=tuple(out_names),
            lowering_input_output_aliases=(),
            sim_require_finite=True,
            sim_require_nnan=True,
            nc=nc,
        ))
        return tuple(outs)

    devices = jax.devices()[:NCORES]
    mesh = Mesh(_np.asarray(devices), ("core",))
    nio = n_params + len(out_names)
    fn = jax.jit(
        shard_map(_body, mesh=mesh,
                  in_specs=(PartitionSpec("core"),) * nio,
                  out_specs=(PartitionSpec("core"),) * len(out_names),
                  check_rep=False),
        donate_argnums=tuple(range(n_params, nio)),
        keep_unused=True,
    )
    return fn, in_names, out_names, out_avals


def _bench_main(bench_reps, reps):
    import json
    import time
    import jax
    rng = np.random.default_rng(0)
    ins = {
        "x": rng.standard_normal((B, S, D)).astype(np.float32),
        "Wq": rng.standard_normal((D, D)).astype(np.float32) * 0.036,
        "bq": rng.standard_normal((D,)).astype(np.float32) * 0.036,
        "Wk": rng.standard_normal((D, D)).astype(np.float32) * 0.036,
        "bk": rng.standard_normal((D,)).astype(np.float32) * 0.036,
        "Wv": rng.standard_normal((D, D)).astype(np.float32) * 0.036,
        "bv": rng.standard_normal((D,)).astype(np.float32) * 0.036,
        "Wo": rng.standard_normal((D, D)).astype(np.float32) * 0.036,
        "bo": rng.standard_normal((D,)).astype(np.float32) * 0.036,
    }
    args = [ins[k] for k in ("x", "Wq", "bq", "Wk", "bk", "Wv", "bv",
                             "Wo", "bo")]
    in_maps = _make_in_maps(*args)
    fn, in_names, out_names, out_avals = _runner(bench_reps)
    concat_in = [np.concatenate([m[k] for m in in_maps], axis=0)
                 for k in in_names]
    dev_in = [jax.device_put(a) for a in concat_in]

    def fresh_zeros():
        return [jax.device_put(
            np.zeros((NCORES * a.shape[0], *a.shape[1:]), a.dtype))
            for a in out_avals]

    out = fn(*dev_in, *fresh_zeros())
    jax.block_until_ready(out)
    ts = []
    for _ in range(reps):
        z = fresh_zeros()
        jax.block_until_ready(z)
        t0 = time.perf_counter()
        out = fn(*dev_in, *z)
        jax.block_until_ready(out)
        ts.append(time.perf_counter() - t0)
    print(json.dumps({"bench_reps": bench_reps,
                      "wall_ns": [t * 1e9 for t in ts],
                      "min_wall_ns": min(ts) * 1e9}))


if __name__ == "__main__":
    import sys
    if "--loop" in sys.argv:
        br = ("loop", int(sys.argv[sys.argv.index("--loop") + 1]))
    elif "--bench" in sys.argv:
        br = int(sys.argv[sys.argv.index("--bench") + 1])
    else:
        br = 1
    rp = int(sys.argv[sys.argv.index("--reps") + 1]) if "--reps" in sys.argv else 4
    _bench_main(br, rp)
